# revision 1
# baseline (speedup 1.0000x reference)
"""Trainium2 Bass kernel for CachedMultiheadAttention (sliding-window + ALiBi).

Sharding: 8 cores = 2 batches x 4 head-quartets. Core c handles batch c//4 and
heads [4*(c%4), 4*(c%4)+4). Each core computes QKV projection for its heads,
banded attention (causal + 512 window + ALiBi), and a partial out-projection
over its heads' 256 embedding columns. Host sums the 4 partials per batch.

v3 (all-bf16, PE-dense, batched normalization):
  - inputs pre-cast to bf16 on host (half the HBM traffic); x^T/w loads split
    into many small DMAs so the first matmul inputs land on many rings fast.
  - V projected directly into natural [t, d] layout (lhsT = x^T block), no PE
    transposes; ones column in vnat gives softmax rowsums via the AV matmul.
  - S^T strips: PE (bf16) -> exp on ACT -> multiply by precomputed band*ALiBi
    bias tile (split across DVE / gpsimd).
  - AV results + rowsum rows staged to SBUF immediately (frees PSUM, no
    deadlock), then ONE plain DVE reciprocal per head-pair on the batched
    [8,512] rowsum tile -- no scalar-table thrash, no custom-DVE ops (which
    silently no-op on this rig).
  - out-projection (bf16) per column group right after its normalize, so PE
    stays dense to the end; partial over 256 local e-rows, host-summed.
"""
import math

import numpy as np
import ml_dtypes

import concourse.bass as bass
import concourse.tile as tile
from concourse import bacc, mybir
from concourse.bass_utils import run_bass_kernel_spmd

F32 = mybir.dt.float32
F32R = mybir.dt.float32r
BF16 = mybir.dt.bfloat16

B, T, E, H, HD, W = 2, 2048, 1024, 16, 64, 512
NCORES = 8
HL = 4                # local heads per core
NT = T // 128         # 16 t-blocks

_CACHE = {}


def _get_slopes(n):
    def p2(m):
        start = 2 ** (-(2 ** (-(math.log2(m) - 3))))
        return [start * start**i for i in range(m)]
    if math.log2(n) % 1 == 0:
        return p2(n)
    c = 2 ** math.floor(math.log2(n))
    return p2(c) + _get_slopes(2 * c)[0::2][: n - c]


def _build():
    nc = bacc.Bacc("TRN2", target_bir_lowering=False, debug=False, num_devices=NCORES)
    xT = nc.dram_tensor("xT", [8, 128, T], BF16, kind="ExternalInput").ap()
    wqkv = nc.dram_tensor("wqkv", [8, 128, 768], BF16, kind="ExternalInput").ap()
    wo = nc.dram_tensor("wo", [2, 128, E], BF16, kind="ExternalInput").ap()
    biasd = nc.dram_tensor("biasd", [HL, 128, 640], BF16, kind="ExternalInput").ap()
    outT = nc.dram_tensor("outT", [8, 128, T], BF16, kind="ExternalOutput").ap()

    with tile.TileContext(nc) as tc:
        with (
            tc.tile_pool(name="singles", bufs=1) as singles,
            tc.tile_pool(name="ptp", bufs=3) as ptp,
            tc.tile_pool(name="sprep", bufs=5) as sprep,
            tc.tile_pool(name="aostp", bufs=2) as aostp,
            tc.tile_pool(name="smallp", bufs=2) as smallp,
            tc.tile_pool(name="evp", bufs=8) as evp,
            tc.tile_pool(name="mm", bufs=4, space="PSUM") as mmp,
            tc.tile_pool(name="aop", bufs=2, space="PSUM") as aop,
            tc.tile_pool(name="bcps", bufs=2, space="PSUM") as bcps,
        ):
            dmae = [nc.sync, nc.scalar, nc.gpsimd]

            # --- one-time loads: first-needed first, split fine so the lead
            # chunks land on many DMA rings in parallel ---
            wqkv_sb = singles.tile([128, 8, 768], BF16)
            xT_sb = singles.tile([128, 8, T], BF16)
            di = 0
            for ec in range(8):
                for q4 in range(4):           # wqkv[ec] in 4 pieces (48 KB)
                    dmae[di % 2].dma_start(
                        wqkv_sb[:, ec, q4 * 192:(q4 + 1) * 192],
                        wqkv[ec, :, q4 * 192:(q4 + 1) * 192])
                    di += 1
                for h2 in range(2):           # x tb0 chunk in 2 pieces (64 KB)
                    dmae[di % 2].dma_start(
                        xT_sb[:, ec, h2 * 256:(h2 + 1) * 256],
                        xT[ec, :, h2 * 256:(h2 + 1) * 256])
                    di += 1
            for tb in range(1, 4):
                for ec in range(8):
                    # gpsimd's slow issuance only for the last-needed chunks
                    eng = nc.gpsimd if tb == 3 else dmae[di % 2]
                    eng.dma_start(
                        xT_sb[:, ec, tb * 512:(tb + 1) * 512],
                        xT[ec, :, tb * 512:(tb + 1) * 512])
                    di += 1
            bias_sb = singles.tile([128, HL, 640], BF16)
            for hl in range(HL):
                dmae[hl % 2].dma_start(bias_sb[:, hl, :], biasd[hl])
            wo_sb = singles.tile([128, 2, E], BF16)
            nc.sync.dma_start(wo_sb[:, 0, :], wo[0])
            nc.scalar.dma_start(wo_sb[:, 1, :], wo[1])

            qkvT = singles.tile([128, 4, T], BF16)   # slots: Qp0 Qp1 Kp0 Kp1
            vnat = singles.tile([128, HL, NT, HD + 1], BF16)
            nc.gpsimd.memset(vnat[:], 1.0)           # ones column at [...,64]
            ao2T = singles.tile([128, 2, T], BF16)   # normalized AO^T
            ones128 = singles.tile([128, 512], F32)
            nc.gpsimd.memset(ones128[:], 1.0)
            rsab = [singles.tile([97, 512], F32, name=f"rsab{x}") for x in range(4)]
            for x in range(4):
                nc.gpsimd.memset(rsab[x][:], 1.0)
            rrab = [singles.tile([97, 512], F32, name=f"rrab{x}") for x in range(4)]
            onesr = singles.tile([1, 64], F32)
            nc.gpsimd.memset(onesr[:], 1.0)

            # --- phase 1: Q^T/K^T projection + V natural-layout projection ---
            for tb in range(4):
                for m in (0, 2, 1, 3):
                    pt = mmp.tile([128, 512], F32, tag="mm512")
                    for ec in range(8):
                        nc.tensor.matmul(
                            pt[:],
                            lhsT=wqkv_sb[:, ec, m * 128:(m + 1) * 128],
                            rhs=xT_sb[:, ec, tb * 512:(tb + 1) * 512],
                            start=(ec == 0), stop=(ec == 7),
                        )
                    if m % 2 == 0:
                        nc.scalar.copy(qkvT[:, m, tb * 512:(tb + 1) * 512], pt[:])
                    else:
                        nc.vector.tensor_copy(
                            qkvT[:, m, tb * 512:(tb + 1) * 512], pt[:])
                for tl in range(4):
                    tk = tb * 4 + tl
                    vt = mmp.tile([128, 4, HD], F32, tag="mm512")
                    for ec in range(8):
                        nc.tensor.matmul(
                            vt[:],
                            lhsT=xT_sb[:, ec, tk * 128:(tk + 1) * 128],
                            rhs=wqkv_sb[:, ec, 512:768],
                            start=(ec == 0), stop=(ec == 7),
                        )
                    nc.vector.tensor_copy(vnat[:, :, tk, 0:HD], vt[:])

            # --- phase 2: attention, head-pair interleaved ---
            # Per sq: S strips -> AV (+staging). sq0's reciprocal/normalize
            # chain is emitted AFTER sq1's strips so the vector queue never
            # blocks the strip pipeline; the broadcast of 1/rowsum is a rank-1
            # PE matmul into PSUM (gpsimd stays single-op-type); phase 3 last.
            pend = {}

            def out_proj_tb(tb):
                for fc in range(8):
                    po = mmp.tile([128, 512], F32, tag="mm512")
                    for c2 in range(2):
                        nc.tensor.matmul(
                            po[:],
                            lhsT=wo_sb[:, c2, fc * 128:(fc + 1) * 128],
                            rhs=ao2T[:, c2, tb * 512:(tb + 1) * 512],
                            start=(c2 == 0), stop=(c2 == 1),
                        )
                    ev = evp.tile([128, 512], BF16, tag="ev")
                    nc.scalar.copy(ev[:], po[:])
                    nc.sync.dma_start(outT[fc, :, tb * 512:(tb + 1) * 512], ev[:])

            def strips(sq, pths):
                for jb in range(NT):
                    nq = min(5, NT - jb)
                    qw = nq * 128
                    w0 = min(qw, 512)
                    for hh in range(2):
                        h = 2 * sq + hh
                        r0 = hh * 64
                        pth = pths[hh]
                        praw = sprep.tile([128, 640], BF16, tag="praw")
                        s5 = mmp.tile([128, 512], F32, tag="mm512")
                        nc.tensor.matmul(
                            s5[:, 0:w0],
                            lhsT=qkvT[r0:r0 + 64, 2 + sq, jb * 128:(jb + 1) * 128],
                            rhs=qkvT[r0:r0 + 64, sq, jb * 128:jb * 128 + w0],
                            start=True, stop=True,
                        )
                        nc.scalar.activation(
                            out=praw[:, 0:w0], in_=s5[:, 0:w0],
                            func=mybir.ActivationFunctionType.Exp,
                        )
                        if qw > 512:
                            s1 = mmp.tile([128, 128], F32, tag="mm512")
                            nc.tensor.matmul(
                                s1[:],
                                lhsT=qkvT[r0:r0 + 64, 2 + sq, jb * 128:(jb + 1) * 128],
                                rhs=qkvT[r0:r0 + 64, sq, jb * 128 + 512:jb * 128 + qw],
                                start=True, stop=True,
                            )
                            nc.scalar.activation(
                                out=praw[:, 512:qw], in_=s1[:],
                                func=mybir.ActivationFunctionType.Exp,
                            )
                        # P = exp(S) * exp(bias): band mask + ALiBi
                        eng = nc.vector if hh == 0 else nc.gpsimd
                        eng.tensor_tensor(
                            out=pth[:, jb, 0:qw], in0=praw[:, 0:qw],
                            in1=bias_sb[:, h, 0:qw], op=mybir.AluOpType.mult,
                        )

            def av_stage(sq, pths):
                aostg = aostp.tile([64, 8, 512], BF16, tag="aostg")
                for g in range(4):
                    for hh in range(2):
                        h = 2 * sq + hh
                        pth = pths[hh]
                        ao = aop.tile([65, 512], F32, tag="ao")
                        jbs = [4 * g] + [jb for jb in range(max(0, 4 * g - 4), 4 * g + 4)
                                         if jb != 4 * g]
                        for i, jb in enumerate(jbs):
                            qb_lo = max(4 * g, jb)
                            qb_hi = min(4 * g + 3, jb + 4)
                            wdt = (qb_hi - qb_lo + 1) * 128
                            ao_off = (qb_lo - 4 * g) * 128
                            p_off = (qb_lo - jb) * 128
                            nc.tensor.matmul(
                                ao[:, ao_off:ao_off + wdt],
                                lhsT=vnat[:, h, jb, :],
                                rhs=pth[:, jb, p_off:p_off + wdt],
                                start=(i == 0), stop=(i == len(jbs) - 1),
                                skip_group_check=True,
                            )
                        nc.vector.tensor_tensor(
                            out=rsab[2 * sq + hh][32 * g:32 * g + 1, :],
                            in0=ao[64:65, :], in1=ones128[0:1, :],
                            op=mybir.AluOpType.mult)
                        if hh == 0:
                            nc.scalar.copy(aostg[:, 2 * g, :], ao[0:64, :])
                        elif sq == 1:
                            nc.scalar.copy(aostg[:, 2 * g + 1, :], ao[0:64, :])
                        else:
                            nc.vector.tensor_copy(aostg[:, 2 * g + 1, :], ao[0:64, :])
                pend[sq] = aostg

            def recips(sq):
                nc.vector.reciprocal(rrab[2 * sq][:], rsab[2 * sq][:])
                nc.vector.reciprocal(rrab[2 * sq + 1][:], rsab[2 * sq + 1][:])

            def chain_norm(sq, phase3=False):
                aostg = pend.pop(sq)
                # per slot: row extract -> rank-1 PE broadcast into PSUM ->
                # normalize multiply (reciprocals already done)
                for g in range(4):
                    for hh in range(2):
                        r0 = hh * 64
                        rrg = smallp.tile([1, 512], F32R, tag="rrg", bufs=3)
                        nc.gpsimd.tensor_tensor(
                            out=rrg[:], in0=rrab[2 * sq + hh][32 * g:32 * g + 1, :],
                            in1=ones128[32 * g:32 * g + 1, :],
                            op=mybir.AluOpType.mult)
                        bc = bcps.tile([64, 512], F32, tag="bc")
                        nc.tensor.matmul(
                            bc[:], lhsT=onesr[:].bitcast(F32R),
                            rhs=rrg[:],
                            start=True, stop=True,
                        )
                        nc.vector.tensor_tensor(
                            out=ao2T[r0:r0 + 64, sq, g * 512:(g + 1) * 512],
                            in0=aostg[:, 2 * g + hh, :], in1=bc[:],
                            op=mybir.AluOpType.mult,
                        )
                    if phase3:
                        out_proj_tb(g)

            pths0 = [ptp.tile([128, NT, 640], BF16, tag="pth", name="ptha0"),
                     ptp.tile([128, NT, 640], BF16, tag="pth", name="pthb0")]
            strips(0, pths0)
            av_stage(0, pths0)
            recips(0)
            pths1 = [ptp.tile([128, NT, 640], BF16, tag="pth", name="ptha1"),
                     ptp.tile([128, NT, 640], BF16, tag="pth", name="pthb1")]
            strips(1, pths1)
            chain_norm(0)
            av_stage(1, pths1)
            recips(1)
            chain_norm(1, phase3=True)


    nc.compile()
    return nc


def _host_inputs(query, in_proj_weight, out_proj_weight):
    """Per-core input maps (numpy only)."""
    slopes = np.asarray(_get_slopes(H), np.float32)
    q32 = np.asarray(query, np.float32)
    w_in = np.asarray(in_proj_weight, np.float32)
    w_out = np.asarray(out_proj_weight, np.float32)

    # band+alibi bias tiles, shift-invariant per head: [h, jj, cc]
    jj = np.arange(128)[:, None]
    cc = np.arange(640)[None, :]
    allowed = (cc >= jj) & (cc - jj <= W)
    in_maps = []
    for c in range(NCORES):
        b, hq = divmod(c, 4)
        heads = np.arange(4 * hq, 4 * hq + HL)
        rows = (heads[:, None] * HD + np.arange(HD)[None, :]).reshape(-1)  # 256 rows
        wq = w_in[rows, :] * (1.0 / math.sqrt(HD))
        wk = w_in[E + rows, :]
        wv = w_in[2 * E + rows, :]
        w_loc = np.concatenate([wq, wk, wv], axis=0)          # [768, E]
        wqkv = np.ascontiguousarray(
            w_loc.T.reshape(8, 128, 768)).astype(ml_dtypes.bfloat16)

        xT = np.ascontiguousarray(
            q32[b].T.reshape(8, 128, T)).astype(ml_dtypes.bfloat16)

        wo_loc = np.ascontiguousarray(
            w_out[:, rows].T.reshape(2, 128, E)).astype(ml_dtypes.bfloat16)

        biasd = np.empty((HL, 128, 640), ml_dtypes.bfloat16)
        for hl in range(HL):
            s = slopes[4 * hq + hl]
            eb = np.where(allowed, np.exp(-s * (cc - jj).astype(np.float64)), 0.0)
            biasd[hl] = eb.astype(ml_dtypes.bfloat16)

        in_maps.append({"xT": xT, "wqkv": wqkv, "wo": wo_loc, "biasd": biasd})
    return in_maps


def _assemble(results):
    out = np.zeros((B, T, E), np.float32)
    for c in range(NCORES):
        b = c // 4
        part = np.asarray(results[c]["outT"]).astype(np.float32)  # [8,128,T]
        out[b] += part.reshape(E, T).T
    return out


def kernel(query, in_proj_weight, out_proj_weight, num_heads, sliding_window_size):
    assert int(num_heads) == H and int(sliding_window_size) == W
    assert query.shape == (B, T, E)
    if "nc" not in _CACHE:
        _CACHE["nc"] = _build()
    in_maps = _host_inputs(query, in_proj_weight, out_proj_weight)
    res = run_bass_kernel_spmd(_CACHE["nc"], in_maps, list(range(NCORES))).results
    return _assemble(res)



# revision 18
# speedup vs baseline: 1.0558x; 1.0558x over previous
"""Trainium2 Bass kernel for CachedMultiheadAttention (sliding-window + ALiBi).

Sharding: 8 cores = 2 batches x 4 head-quartets. Core c handles batch c//4 and
heads [4*(c%4), 4*(c%4)+4). Each core computes QKV projection for its heads,
banded attention (causal + 512 window + ALiBi), and a partial out-projection
over its heads' 256 embedding columns. Host sums the 4 partials per batch.

v4 (pipelined emission, engine rebalance):
  - wqkv columns reordered [Qp0|Kp0|Qp1|Kp1|Vp0|Vp1] so pair-0 Q/K loads +
    projects first; strips(0) starts ~25us earlier than v3.
  - V / QK1 projections emitted as PE filler inside pair-0's attention phase;
    out-proj for groups 0-2 emitted inside pair-1's phase (per-group recip
    halves) so the tail is only group 3's chain.
  - 640-wide PSUM strip tiles -> ONE exp per strip (ACT is the phase-2
    co-bottleneck; this cuts its op count by a third).
  - reciprocal via ACT Ln -> Exp(scale=-1) on batched [8,512] rowsum tiles
    (DVE reciprocal is ~6cy/elem = 3.3us per 512-wide op).
  - rowsum-reciprocal rows fanned out to [1,512] tiles via SBUF->SBUF DMA on
    the sync queue (frees DVE), then broadcast via a K=1 bf16 PE matmul
    (replaces v3's fp32r broadcast + slow gpsimd extracts).
  - scalar engine does exp only (plus a few early copies); PSUM->SBUF copies
    spread across DVE/Pool; all input DMAs issued from scalar/sync/gpsimd
    before compute needs them; all output stores on sync.
"""
import math

import numpy as np
import ml_dtypes

import concourse.bass as bass
import concourse.tile as tile
from concourse import bacc, mybir
from concourse.bass_utils import run_bass_kernel_spmd

F32 = mybir.dt.float32
BF16 = mybir.dt.bfloat16

B, T, E, H, HD, W = 2, 2048, 1024, 16, 64, 512
NCORES = 8
HL = 4                # local heads per core
NT = T // 128         # 16 t-blocks

_CACHE = {}


def _get_slopes(n):
    def p2(m):
        start = 2 ** (-(2 ** (-(math.log2(m) - 3))))
        return [start * start**i for i in range(m)]
    if math.log2(n) % 1 == 0:
        return p2(n)
    c = 2 ** math.floor(math.log2(n))
    return p2(c) + _get_slopes(2 * c)[0::2][: n - c]


def _build():
    nc = bacc.Bacc("TRN2", target_bir_lowering=False, debug=False, num_devices=NCORES)
    xT = nc.dram_tensor("xT", [8, 128, T], BF16, kind="ExternalInput").ap()
    wqkv = nc.dram_tensor("wqkv", [8, 128, 768], BF16, kind="ExternalInput").ap()
    wo = nc.dram_tensor("wo", [2, 128, E], BF16, kind="ExternalInput").ap()
    biasd = nc.dram_tensor("biasd", [HL, 128, 640], BF16, kind="ExternalInput").ap()
    outT = nc.dram_tensor("outT", [8, 128, T], BF16, kind="ExternalOutput").ap()

    with tile.TileContext(nc) as tc:
        with (
            tc.tile_pool(name="singles", bufs=1) as singles,
            tc.tile_pool(name="ptp", bufs=3) as ptp,
            tc.tile_pool(name="sprep", bufs=4) as sprep,
            tc.tile_pool(name="smallp", bufs=3) as smallp,
            tc.tile_pool(name="evp", bufs=4) as evp,
            tc.tile_pool(name="mm", bufs=2, space="PSUM") as mmp,
            tc.tile_pool(name="sp", bufs=2, space="PSUM") as spp,
            tc.tile_pool(name="aop", bufs=2, space="PSUM") as aop,
        ):
            # ---------------- input DMAs ----------------
            wqkv_sb = singles.tile([128, 8, 768], BF16)
            xT_sb = singles.tile([128, 8, T], BF16)
            bias_sb = singles.tile([128, HL, 640], BF16)
            wo_sb = singles.tile([128, 2, E], BF16)

            # scalar queue: QK0 weight cols first, then wo + bias
            for ec in range(8):
                nc.scalar.dma_start(wqkv_sb[:, ec, 0:256], wqkv[ec, :, 0:256])
            nc.scalar.dma_start(wo_sb[:, 0, :], wo[0])
            nc.scalar.dma_start(wo_sb[:, 1, :], wo[1])
            for hl in range(HL):
                nc.scalar.dma_start(bias_sb[:, hl, :], biasd[hl])
            # sync queue: xT tb0, tb1
            for ec in range(8):
                nc.sync.dma_start(xT_sb[:, ec, 0:512], xT[ec, :, 0:512])
            for ec in range(8):
                nc.sync.dma_start(xT_sb[:, ec, 512:1024], xT[ec, :, 512:1024])
            # gpsimd queue: rest of wqkv (QK1+V cols), xT tb2-3
            vnat = singles.tile([128, HL, NT, HD + 1], BF16)
            nc.gpsimd.memset(vnat[:, :, :, HD:HD + 1], 1.0)
            onesbf = singles.tile([1, 64], BF16)
            nc.gpsimd.memset(onesbf[:], 1.0)
            ones512 = singles.tile([1, 512], F32)
            nc.gpsimd.memset(ones512[:], 1.0)
            for ec in range(8):
                nc.gpsimd.dma_start(wqkv_sb[:, ec, 256:768], wqkv[ec, :, 256:768])
            for ec in range(8):
                nc.gpsimd.dma_start(xT_sb[:, ec, 1024:2048], xT[ec, :, 1024:2048])

            qkvT = singles.tile([128, 4, T], BF16)   # slots: Qp0 Qp1 Kp0 Kp1
            ao2T = singles.tile([128, 2, T], BF16)   # normalized AO^T
            aostg = [singles.tile([64, 8, 512], BF16, name=f"aostg{p}")
                     for p in range(2)]
            # rowsum rows live at partition 32*g (DVE partition bases must be
            # 32-aligned); free dims = [hh, 512]
            rs8 = [singles.tile([97, 2, 512], F32, name=f"rs8_{p}")
                   for p in range(2)]
            lnt = singles.tile([97, 2, 512], F32)
            rr8 = [singles.tile([97, 2, 512], BF16, name=f"rr8_{p}")
                   for p in range(2)]

            # wqkv col map: slot0=Q pair0 @0:128, slot2=K pair0 @128:256,
            #               slot1=Q pair1 @256:384, slot3=K pair1 @384:512,
            #               V pair0 @512:640, V pair1 @640:768
            SLOT_COLS = {0: (0, 128), 2: (128, 256), 1: (256, 384), 3: (384, 512)}

            def qk_chain(slot, tb, copy_eng):
                c0, c1 = SLOT_COLS[slot]
                pt = mmp.tile([128, 512], F32, tag="mm512")
                for ec in range(8):
                    nc.tensor.matmul(
                        pt[:],
                        lhsT=wqkv_sb[:, ec, c0:c1],
                        rhs=xT_sb[:, ec, tb * 512:(tb + 1) * 512],
                        start=(ec == 0), stop=(ec == 7),
                    )
                if copy_eng is nc.scalar:
                    nc.scalar.copy(qkvT[:, slot, tb * 512:(tb + 1) * 512], pt[:])
                else:
                    copy_eng.tensor_copy(qkvT[:, slot, tb * 512:(tb + 1) * 512], pt[:])

            def v_chain(p, tk, cast_eng):
                vt = mmp.tile([128, 2, HD], F32, tag="mm512")
                for ec in range(8):
                    nc.tensor.matmul(
                        vt[:],
                        lhsT=xT_sb[:, ec, tk * 128:(tk + 1) * 128],
                        rhs=wqkv_sb[:, ec, 512 + 128 * p:640 + 128 * p],
                        start=(ec == 0), stop=(ec == 7),
                    )
                dst = vnat[:, 2 * p:2 * p + 2, tk, 0:HD]
                if cast_eng is nc.scalar:
                    nc.scalar.copy(dst, vt[:])
                else:
                    cast_eng.tensor_copy(dst, vt[:])

            def strip(sq, jb, hh, pth):
                h = 2 * sq + hh
                r0 = hh * 64
                qw = min(5, NT - jb) * 128
                w0 = min(qw, 512)
                s = spp.tile([128, 640], F32, tag="s640")
                nc.tensor.matmul(
                    s[:, 0:w0],
                    lhsT=qkvT[r0:r0 + 64, 2 + sq, jb * 128:(jb + 1) * 128],
                    rhs=qkvT[r0:r0 + 64, sq, jb * 128:jb * 128 + w0],
                    start=True, stop=True,
                )
                if qw > 512:
                    nc.tensor.matmul(
                        s[:, 512:qw],
                        lhsT=qkvT[r0:r0 + 64, 2 + sq, jb * 128:(jb + 1) * 128],
                        rhs=qkvT[r0:r0 + 64, sq, jb * 128 + 512:jb * 128 + qw],
                        start=True, stop=True,
                    )
                praw = sprep.tile([128, 640], BF16, tag="praw")
                nc.scalar.activation(
                    out=praw[:, 0:qw], in_=s[:, 0:qw],
                    func=mybir.ActivationFunctionType.Exp,
                )
                # SBUF-only op: Pool takes the bigger share (DVE carries all
                # the PSUM->SBUF copies, which Pool cannot do)
                eng = nc.vector if hh == 0 else nc.gpsimd
                eng.tensor_tensor(
                    out=pth[:, jb, 0:qw], in0=praw[:, 0:qw],
                    in1=bias_sb[:, h, 0:qw], op=mybir.AluOpType.mult,
                )

            def av_group(sq, g, pths, stage_engs):
                for hh in range(2):
                    h = 2 * sq + hh
                    slot = 2 * g + hh
                    ao = aop.tile([65, 512], F32, tag="ao")
                    jbs = [4 * g] + [jb for jb in range(max(0, 4 * g - 4), 4 * g + 4)
                                     if jb != 4 * g]
                    for i, jb in enumerate(jbs):
                        qb_lo = max(4 * g, jb)
                        qb_hi = min(4 * g + 3, jb + 4)
                        wdt = (qb_hi - qb_lo + 1) * 128
                        ao_off = (qb_lo - 4 * g) * 128
                        p_off = (qb_lo - jb) * 128
                        nc.tensor.matmul(
                            ao[:, ao_off:ao_off + wdt],
                            lhsT=vnat[:, h, jb, :],
                            rhs=pths[hh][:, jb, p_off:p_off + wdt],
                            start=(i == 0), stop=(i == len(jbs) - 1),
                            skip_group_check=True,
                        )
                    nc.vector.tensor_tensor(
                        out=rs8[sq][32 * g:32 * g + 1, hh, :], in0=ao[64:65, :],
                        in1=ones512[:], op=mybir.AluOpType.mult)
                    stage_engs[hh].tensor_copy(aostg[sq][:, slot, :], ao[0:64, :])

            def recip(sq, lo, hi):
                # lo/hi are partition bounds (32-aligned); valid rows at 32*g
                nc.scalar.activation(
                    out=lnt[lo:hi, :, :], in_=rs8[sq][lo:hi, :, :],
                    func=mybir.ActivationFunctionType.Ln,
                )
                nc.scalar.activation(
                    out=rr8[sq][lo:hi, :, :], in_=lnt[lo:hi, :, :],
                    func=mybir.ActivationFunctionType.Exp, scale=-1.0,
                )

            def norm_group(sq, g):
                for hh in range(2):
                    slot = 2 * g + hh
                    r0 = hh * 64
                    rrg = smallp.tile([1, 512], BF16, tag="rrg")
                    nc.vector.tensor_copy(rrg[:], rr8[sq][32 * g:32 * g + 1, hh, :])
                    bc = mmp.tile([64, 512], F32, tag="mm512")
                    nc.tensor.matmul(
                        bc[:], lhsT=onesbf[:], rhs=rrg[:], start=True, stop=True,
                    )
                    nc.vector.tensor_tensor(
                        out=ao2T[r0:r0 + 64, sq, g * 512:(g + 1) * 512],
                        in0=aostg[sq][:, slot, :], in1=bc[:],
                        op=mybir.AluOpType.mult,
                    )

            def out_proj_tb(tb, ev_engs):
                for fc in range(8):
                    po = mmp.tile([128, 512], F32, tag="mm512")
                    for c2 in range(2):
                        nc.tensor.matmul(
                            po[:],
                            lhsT=wo_sb[:, c2, fc * 128:(fc + 1) * 128],
                            rhs=ao2T[:, c2, tb * 512:(tb + 1) * 512],
                            start=(c2 == 0), stop=(c2 == 1),
                        )
                    ev = evp.tile([128, 512], BF16, tag="ev")
                    eng = ev_engs[fc % len(ev_engs)]
                    if eng is nc.scalar:
                        nc.scalar.copy(ev[:], po[:])
                    else:
                        eng.tensor_copy(ev[:], po[:])
                    nc.sync.dma_start(outT[fc, :, tb * 512:(tb + 1) * 512], ev[:])

            # ---------------- phase 1a: QK0 ----------------
            qk0_copy = [nc.scalar, nc.vector, nc.scalar, nc.vector,
                        nc.scalar, nc.vector, nc.scalar, nc.vector]
            i = 0
            for tb in range(4):
                for slot in (0, 2):
                    qk_chain(slot, tb, qk0_copy[i]); i += 1
            # Vp0 for AV(0,g0)
            for tk in range(4):
                v_chain(0, tk, nc.scalar)

            # ---------------- pair 0 attention + fillers ----------------
            pths0 = [ptp.tile([128, NT, 640], BF16, tag="pth", name="ptha0"),
                     ptp.tile([128, NT, 640], BF16, tag="pth", name="pthb0")]
            # filler work units emitted between strip groups: remaining Vp0,
            # Vp1, and all of QK1 (needed before strips(1)).
            fillers = []
            for tk in range(4, 16):
                fillers.append(("v", 0, tk))
            for tk in range(16):
                fillers.append(("v", 1, tk))
            for tb in range(4):
                for slot in (1, 3):
                    fillers.append(("qk", slot, tb))
            fi = 0
            v_engs = [nc.vector, nc.vector]
            qk1_engs = [nc.vector, nc.vector]

            def drain_fillers(n):
                nonlocal fi
                for _ in range(n):
                    if fi >= len(fillers):
                        return
                    kind, a, b = fillers[fi]
                    if kind == "v":
                        v_chain(a, b, v_engs[fi % 2])
                    else:
                        qk_chain(a, b, qk1_engs[fi % 2])
                    fi += 1

            stage0 = [nc.vector, nc.vector]
            for g in range(4):
                for jb in range(4 * g, 4 * g + 4):
                    strip(0, jb, 0, pths0[0])
                    strip(0, jb, 1, pths0[1])
                    drain_fillers(2)
                av_group(0, g, pths0, stage0)
                drain_fillers(2)
            recip(0, 0, 97)

            # ---------------- pair 1 attention + pair-0 norms + early out-proj
            pths1 = [ptp.tile([128, NT, 640], BF16, tag="pth", name="ptha1"),
                     ptp.tile([128, NT, 640], BF16, tag="pth", name="pthb1")]
            ev_mid = [nc.vector, nc.scalar]
            for g in range(4):
                for jb in range(4 * g, 4 * g + 4):
                    strip(1, jb, 0, pths1[0])
                    strip(1, jb, 1, pths1[1])
                    drain_fillers(3)
                av_group(1, g, pths1, stage0)
                # pair-0 normalization interleaves with pair-1 strips
                norm_group(0, g)
                if g == 1:
                    recip(1, 0, 33)
                if g == 2:
                    norm_group(1, 0)
                    out_proj_tb(0, ev_mid)
                if g == 3:
                    norm_group(1, 1)
                    out_proj_tb(1, ev_mid)
            drain_fillers(len(fillers))
            recip(1, 64, 97)
            ev_tail = [nc.scalar, nc.vector]
            norm_group(1, 2)
            out_proj_tb(2, ev_tail)
            norm_group(1, 3)
            out_proj_tb(3, ev_tail)

    nc.compile()
    return nc


def _host_inputs(query, in_proj_weight, out_proj_weight):
    """Per-core input maps (numpy only)."""
    slopes = np.asarray(_get_slopes(H), np.float32)
    q32 = np.asarray(query, np.float32)
    w_in = np.asarray(in_proj_weight, np.float32)
    w_out = np.asarray(out_proj_weight, np.float32)

    # band+alibi bias tiles, shift-invariant per head: [h, jj, cc]
    jj = np.arange(128)[:, None]
    cc = np.arange(640)[None, :]
    allowed = (cc >= jj) & (cc - jj <= W)
    in_maps = []
    for c in range(NCORES):
        b, hq = divmod(c, 4)
        heads = np.arange(4 * hq, 4 * hq + HL)
        rows = (heads[:, None] * HD + np.arange(HD)[None, :]).reshape(-1)  # 256
        p0 = rows[0:128]
        p1 = rows[128:256]
        sc = 1.0 / math.sqrt(HD)
        # cols: [Qp0 | Kp0 | Qp1 | Kp1 | Vp0 | Vp1]
        w_loc = np.concatenate([
            w_in[p0, :] * sc, w_in[E + p0, :],
            w_in[p1, :] * sc, w_in[E + p1, :],
            w_in[2 * E + p0, :], w_in[2 * E + p1, :],
        ], axis=0)                                            # [768, E]
        wqkv = np.ascontiguousarray(
            w_loc.T.reshape(8, 128, 768)).astype(ml_dtypes.bfloat16)

        xT = np.ascontiguousarray(
            q32[b].T.reshape(8, 128, T)).astype(ml_dtypes.bfloat16)

        wo_loc = np.ascontiguousarray(
            w_out[:, rows].T.reshape(2, 128, E)).astype(ml_dtypes.bfloat16)

        biasd = np.empty((HL, 128, 640), ml_dtypes.bfloat16)
        for hl in range(HL):
            s = slopes[4 * hq + hl]
            eb = np.where(allowed, np.exp(-s * (cc - jj).astype(np.float64)), 0.0)
            biasd[hl] = eb.astype(ml_dtypes.bfloat16)

        in_maps.append({"xT": xT, "wqkv": wqkv, "wo": wo_loc, "biasd": biasd})
    return in_maps


def _assemble(results):
    out = np.zeros((B, T, E), np.float32)
    for c in range(NCORES):
        b = c // 4
        part = np.asarray(results[c]["outT"]).astype(np.float32)  # [8,128,T]
        out[b] += part.reshape(E, T).T
    return out


def kernel(query, in_proj_weight, out_proj_weight, num_heads, sliding_window_size):
    assert int(num_heads) == H and int(sliding_window_size) == W
    assert query.shape == (B, T, E)
    if "nc" not in _CACHE:
        _CACHE["nc"] = _build()
    in_maps = _host_inputs(query, in_proj_weight, out_proj_weight)
    res = run_bass_kernel_spmd(_CACHE["nc"], in_maps, list(range(NCORES))).results
    return _assemble(res)
